# revision 1
# baseline (speedup 1.0000x reference)
"""GRNN regressor on 8 TRN2 NeuronCores.

Math: w[b,n] = exp(-(||x_b||^2 + ||t_n||^2 - 2 x_b.t_n)/2); out[b] = (w@y)/(w@1).

Strategy (matches the sharding hint): X_train/y_train sharded over N across
8 cores; x replicated. Per core, one matmul with an augmented feature dim
(K=66: 64 features + both squared-norm terms) produces -dist^2/2 directly in
PSUM with train-points on partitions; ScalarE Exp turns it into weights; a
second PSUM-accumulating matmul against [y, 1] contracts over train points,
yielding per-core partial [numerator; denominator] of shape [2, 4096].
The all-reduce over cores plus the final divide happen on host (32KB/core).
"""

import numpy as np

B, F, N, P = 4096, 64, 100000, 8
NS = N // P            # 12500 train points per core
NB = 128               # train-point block (PSUM partitions)
NSP = ((NS + NB - 1) // NB) * NB   # 12544 padded
NBLK = NSP // NB       # 98
BBLK = 512             # query block (moving free dim / PSUM bank)
K = F + 2              # augmented contraction dim

_cache = {}


def _build():
    import concourse.bacc as bacc
    import concourse.mybir as mybir
    import concourse.tile as tile

    dt = mybir.dt.float32
    nc = bacc.Bacc("TRN2", target_bir_lowering=False, debug=False)
    xa_d = nc.dram_tensor("xa", [K, B], dt, kind="ExternalInput")
    ta_d = nc.dram_tensor("ta", [K, NSP], dt, kind="ExternalInput")
    y1_d = nc.dram_tensor("y1", [NB, 2 * NBLK], dt, kind="ExternalInput")
    out_d = nc.dram_tensor("out", [2, B], dt, kind="ExternalOutput")

    with tile.TileContext(nc) as tc:
        with (
            tc.tile_pool(name="const", bufs=1) as cpool,
            tc.tile_pool(name="w", bufs=3) as wpool,
            tc.tile_pool(name="res", bufs=2) as rpool,
            tc.tile_pool(name="ps", bufs=3, space="PSUM") as spool,
            tc.tile_pool(name="pacc", bufs=2, space="PSUM") as apool,
        ):
            xa = cpool.tile([K, B], dt)
            ta = cpool.tile([K, NSP], dt)
            y1 = cpool.tile([NB, 2 * NBLK], dt)
            zb = cpool.tile([NB, 1], dt)
            nc.sync.dma_start(xa[:], xa_d[:])
            nc.sync.dma_start(ta[:], ta_d[:])
            nc.sync.dma_start(y1[:], y1_d[:])
            nc.gpsimd.memset(zb[:], 0.0)

            for b in range(B // BBLK):
                acc = apool.tile([2, BBLK], dt)
                xsl = xa[:, b * BBLK : (b + 1) * BBLK]
                for ni in range(NBLK):
                    s = spool.tile([NB, BBLK], dt)
                    nc.tensor.matmul(
                        s[:], ta[:, ni * NB : (ni + 1) * NB], xsl,
                        start=True, stop=True,
                    )
                    w = wpool.tile([NB, BBLK], dt)
                    nc.scalar.activation(
                        w[:], s[:], mybir.ActivationFunctionType.Exp, bias=zb[:]
                    )
                    nc.tensor.matmul(
                        acc[:], y1[:, 2 * ni : 2 * ni + 2], w[:],
                        start=(ni == 0), stop=(ni == NBLK - 1),
                    )
                res = rpool.tile([2, BBLK], dt)
                nc.vector.tensor_copy(res[:], acc[:])
                nc.sync.dma_start(out_d[:, b * BBLK : (b + 1) * BBLK], res[:])

    nc.compile()
    return nc


def kernel(x, X_train, y_train):
    from concourse.bass_utils import run_bass_kernel_spmd

    x = np.asarray(x, np.float32)
    X_train = np.asarray(X_train, np.float32)
    y_train = np.asarray(y_train, np.float32)

    xa = np.empty((K, B), np.float32)
    xa[:F] = x.T
    xa[F] = -0.5 * np.sum(x * x, axis=1)
    xa[F + 1] = 1.0

    in_maps = []
    for c in range(P):
        t = X_train[c * NS : (c + 1) * NS]
        ta = np.zeros((K, NSP), np.float32)
        ta[:F, :NS] = t.T
        ta[F, :] = 1.0
        ta[F + 1, :NS] = -0.5 * np.sum(t * t, axis=1)
        ta[F + 1, NS:] = -1e30  # pad columns get weight exp(-inf) = 0
        y1 = np.zeros((NB, 2 * NBLK), np.float32)
        yc = np.zeros(NSP, np.float32)
        yc[:NS] = y_train[c * NS : (c + 1) * NS]
        y1[:, 0::2] = yc.reshape(NBLK, NB).T
        y1[:, 1::2] = 1.0
        in_maps.append({"xa": xa, "ta": ta, "y1": y1})

    if "nc" not in _cache:
        _cache["nc"] = _build()
    res = run_bass_kernel_spmd(_cache["nc"], in_maps, core_ids=list(range(P)))
    parts = np.stack([np.asarray(r["out"]) for r in res.results])  # [P, 2, B]
    tot = parts.sum(axis=0, dtype=np.float64)
    return (tot[0] / tot[1]).astype(np.float32)



# revision 2
# speedup vs baseline: 1047.5898x; 1047.5898x over previous
"""GRNN regressor on 8 TRN2 NeuronCores.

Math: w[b,n] = exp(-(||x_b||^2 + ||t_n||^2 - 2 x_b.t_n)/2); out[b] = (w@y)/(w@1).

Strategy (matches the sharding hint): X_train/y_train sharded over N across
8 cores; x replicated. Per core, one matmul with an augmented feature dim
(K=66: 64 features + both squared-norm terms) produces -dist^2/2 directly in
PSUM with train-points on partitions; ScalarE Exp turns it into bf16 weights;
a second PSUM-accumulating matmul against [y, 1] contracts over train points,
yielding per-core partial [numerator; denominator] of shape [2, 4096].
The all-reduce over cores plus the final divide happen on host (32KB/core).

Perf notes vs the naive version:
- float32r matmul dtype: 1 cycle/row at moving-dim 512 (plain fp32 is 4).
- Exp output and the [y,1] operand are bf16 so the second matmul also runs
  at 1 cycle/row.
- Activations are batched over G=3 PSUM banks (free dim 1536) to amortize
  the ~185ns fixed Act-engine access latency per instruction.
"""

import numpy as np

B, F, N, P = 4096, 64, 100000, 8
NS = N // P            # 12500 train points per core
NB = 128               # train-point block (PSUM partitions)
NSP = ((NS + NB - 1) // NB) * NB   # 12544 padded
NBLK = NSP // NB       # 98
BBLK = 512             # query block (moving free dim / PSUM bank)
K = F + 2              # augmented contraction dim
G = 3                  # PSUM banks per activation batch

_cache = {}


def _build(reps=1):
    import concourse.bacc as bacc
    import concourse.mybir as mybir
    import concourse.tile as tile

    f32 = mybir.dt.float32
    f32r = mybir.dt.float32r
    bf16 = mybir.dt.bfloat16
    nc = bacc.Bacc("TRN2", target_bir_lowering=False, debug=False)
    xa_d = nc.dram_tensor("xa", [K, B], f32r, kind="ExternalInput")
    ta_d = nc.dram_tensor("ta", [K, NSP], f32r, kind="ExternalInput")
    y1_d = nc.dram_tensor("y1", [NB, 2 * NBLK], bf16, kind="ExternalInput")
    out_d = nc.dram_tensor("out", [2, B], f32, kind="ExternalOutput")

    with tile.TileContext(nc) as tc:
        with (
            tc.tile_pool(name="const", bufs=1) as cpool,
            tc.tile_pool(name="w", bufs=3) as wpool,
            tc.tile_pool(name="res", bufs=2) as rpool,
            tc.tile_pool(name="ps", bufs=2, space="PSUM") as spool,
            tc.tile_pool(name="pacc", bufs=2, space="PSUM") as apool,
        ):
            xa = cpool.tile([K, B], f32r)
            ta = cpool.tile([K, NSP], f32r)
            y1 = cpool.tile([NB, 2 * NBLK], bf16)
            nc.sync.dma_start(xa[:], xa_d[:])
            nc.sync.dma_start(ta[:], ta_d[:])
            nc.sync.dma_start(y1[:], y1_d[:])

            groups = []
            ni = 0
            while ni < NBLK:
                g = min(G, NBLK - ni)
                groups.append((ni, g))
                ni += g

            for _ in range(reps):
                for b in range(B // BBLK):
                    acc = apool.tile([2, BBLK], f32)
                    xsl = xa[:, b * BBLK : (b + 1) * BBLK]
                    for ni, g in groups:
                        s = spool.tile([NB, G * BBLK], f32)
                        for j in range(g):
                            nc.tensor.matmul(
                                s[:, j * BBLK : (j + 1) * BBLK],
                                ta[:, (ni + j) * NB : (ni + j + 1) * NB],
                                xsl,
                                start=True, stop=True,
                            )
                        w = wpool.tile([NB, G * BBLK], bf16)
                        nc.scalar.activation(
                            w[:, : g * BBLK], s[:, : g * BBLK],
                            mybir.ActivationFunctionType.Exp,
                        )
                        for j in range(g):
                            nc.tensor.matmul(
                                acc[:],
                                y1[:, 2 * (ni + j) : 2 * (ni + j) + 2],
                                w[:, j * BBLK : (j + 1) * BBLK],
                                start=(ni + j == 0), stop=(ni + j == NBLK - 1),
                            )
                    res = rpool.tile([2, BBLK], f32)
                    nc.vector.tensor_copy(res[:], acc[:])
                    nc.sync.dma_start(out_d[:, b * BBLK : (b + 1) * BBLK], res[:])

    nc.compile()
    return nc


def _prep_inputs(x, X_train, y_train):
    import ml_dtypes

    x = np.asarray(x, np.float32)
    X_train = np.asarray(X_train, np.float32)
    y_train = np.asarray(y_train, np.float32)

    xa = np.empty((K, B), np.float32)
    xa[:F] = x.T
    xa[F] = -0.5 * np.sum(x * x, axis=1)
    xa[F + 1] = 1.0

    in_maps = []
    for c in range(P):
        t = X_train[c * NS : (c + 1) * NS]
        ta = np.zeros((K, NSP), np.float32)
        ta[:F, :NS] = t.T
        ta[F, :] = 1.0
        ta[F + 1, :NS] = -0.5 * np.sum(t * t, axis=1)
        ta[F + 1, NS:] = -1e30  # pad columns get weight exp(-inf) = 0
        y1 = np.zeros((NB, 2 * NBLK), ml_dtypes.bfloat16)
        yc = np.zeros(NSP, np.float32)
        yc[:NS] = y_train[c * NS : (c + 1) * NS]
        y1[:, 0::2] = yc.reshape(NBLK, NB).T.astype(ml_dtypes.bfloat16)
        y1[:, 1::2] = 1.0
        in_maps.append({"xa": xa, "ta": ta, "y1": y1})
    return in_maps


def kernel(x, X_train, y_train):
    from concourse.bass_utils import run_bass_kernel_spmd

    in_maps = _prep_inputs(x, X_train, y_train)
    if "nc" not in _cache:
        _cache["nc"] = _build()
    res = run_bass_kernel_spmd(_cache["nc"], in_maps, core_ids=list(range(P)))
    parts = np.stack([np.asarray(r["out"]) for r in res.results])  # [P, 2, B]
    tot = parts.sum(axis=0, dtype=np.float64)
    return (tot[0] / tot[1]).astype(np.float32)
